# revision 24
# baseline (speedup 1.0000x reference)
"""GAT layer (GATConv + LayerNorm + residual + ELU) as a distributed Bass kernel
on 8 Trainium2 NeuronCores.

Distribution strategy (graph/data parallel, per sharding hint):
  - Nodes are partitioned across the 8 cores. Core c owns a 6272-node range;
    inputs are rotated per-core so each core's own nodes are local rows
    0..6271. Small params (W, att, ln, res) are replicated.
  - Edges are assigned to the core owning their destination node, sorted by
    destination, so segment softmax + scatter-add stay core-local.
  - The "halo gather" of remote source features: every core computes the full
    h table (cheap matmul, replicated) into its own DRAM, then per-edge rows
    are fetched with dma_gather.

Algorithmic notes:
  - att_src/att_dst dot products are folded into extra columns of W, so
    a_src/a_dst come out of the same matmul that produces h.
  - Segment softmax is computed without the max-subtraction (logits are O(5),
    exp() cannot overflow in f32; result is mathematically identical) and the
    per-edge normalization is folded out:  out[n] = sum_e w_e h_src / sum_e w_e.
    Both numerator and denominator accumulate in one PSUM matmul by appending
    w columns to the message.
  - Scatter-add = TensorE matmul with an on-chip one-hot matrix built by
    is_equal(iota, dst_rel); edges are grouped per 128-dst-block so PSUM
    accumulates each block across its (padded, fixed) number of 128-edge
    subtiles.
"""
import os
import sys

sys.path.insert(0, "/opt/trn_rl_repo")

import numpy as np

import concourse.bass as bass
import concourse.bacc as bacc
import concourse.tile as tile
from concourse import mybir
from concourse.bass_utils import run_bass_kernel_spmd

# ---- problem constants (hardcoded per contract) ----
N = 50000
E = 800000
IN_DIM = 128
OUT_DIM = 256
HEADS = 8
C = OUT_DIM // HEADS
NEG_SLOPE = 0.2
LN_EPS = 1e-5

CORES = 8
P = 128
NLOC = 6272                   # own nodes per core (49 blocks of 128)
NPAD = NLOC * CORES           # 50176 padded node count
NBLK = NLOC // P              # 49 dst blocks per core
TA_ROWS = 32768               # gather table A rows (int16 index limit)
TB_ROWS = NPAD - TA_ROWS      # 17408
ROW = 384                     # bf16 elems/row: [h:256 | a_src hi:8 | a_src lo:8 | pad]
ADROWS = NLOC + P             # a_dst table rows; rows >= NLOC hold -1e4 (pad kill)
ADCOL = 128                   # bf16 per a_dst row (256B): [hi:8 | lo:8 | pad]
PAD_DST = NLOC                # dst_local for padding slots -> a_dst = -1e4 -> w = 0

FP = mybir.dt.float32
BF = mybir.dt.bfloat16
I16 = mybir.dt.int16


def _f32_to_bf16_bits(x: np.ndarray) -> np.ndarray:
    b = np.ascontiguousarray(x, dtype=np.float32).view(np.uint32)
    return ((b + 0x7FFF + ((b >> 16) & 1)) >> 16).astype(np.uint16)


def _wrap_idx(vals: np.ndarray, n_slots: int) -> np.ndarray:
    """dma_gather index layout: slot i -> [i % 16, i // 16], replicated to 128
    partitions."""
    assert n_slots % 16 == 0
    w = np.zeros((16, n_slots // 16), dtype=np.int16)
    w[vals.size and 0 or 0] = w[0]  # noop to keep shape checkers quiet
    if vals.size:
        w[np.arange(vals.size) % 16, np.arange(vals.size) // 16] = vals.astype(np.int16)
    return np.tile(w, (8, 1))


def prepare_host(inputs: dict) -> tuple[dict, list[dict]]:
    """Shard/reindex on host. Returns (meta, per-core input maps)."""
    x = np.asarray(inputs["x"], dtype=np.float32)
    edge_index = np.asarray(inputs["edge_index"])
    W = np.asarray(inputs["W"], dtype=np.float32)            # [128, 256]
    att_src = np.asarray(inputs["att_src"], dtype=np.float32)  # [8, 32]
    att_dst = np.asarray(inputs["att_dst"], dtype=np.float32)
    bias = np.asarray(inputs["bias"], dtype=np.float32)
    ln_gamma = np.asarray(inputs["ln_gamma"], dtype=np.float32)
    ln_beta = np.asarray(inputs["ln_beta"], dtype=np.float32)
    res_W = np.asarray(inputs["res_W"], dtype=np.float32)    # [256, 128]
    res_b = np.asarray(inputs["res_b"], dtype=np.float32)

    # fold attention vectors into extra weight columns
    W3 = W.reshape(IN_DIM, HEADS, C)
    Wsrc = np.einsum("ihc,hc->ih", W3, att_src)              # [128, 8]
    Wdst = np.einsum("ihc,hc->ih", W3, att_dst)
    W_ext = np.concatenate([W, Wsrc], axis=1)                # [128, 264]
    W2 = np.concatenate([Wdst, res_W.T], axis=1)             # [128, 264]

    has_bias = bool(np.any(bias != 0.0))
    has_gamma = bool(np.any(ln_gamma != 1.0))
    has_beta = bool(np.any(ln_beta != 0.0))
    has_resb = bool(np.any(res_b != 0.0))

    xT = np.zeros((IN_DIM, NPAD), dtype=np.float32)
    xT[:, :N] = x.T

    # edges + self loops, assigned by destination owner
    src = np.concatenate([edge_index[0], np.arange(N, dtype=np.int64)]).astype(np.int64)
    dst = np.concatenate([edge_index[1], np.arange(N, dtype=np.int64)]).astype(np.int64)
    owner = dst // NLOC

    # per (core, block) edge lists split by src table half (int16 index limit)
    per_core = []
    for c in range(CORES):
        sel = owner == c
        s_l = (src[sel] - c * NLOC) % NPAD
        d_l = dst[sel] - c * NLOC
        order = np.argsort(d_l, kind="stable")
        s_l, d_l = s_l[order], d_l[order]
        blk = d_l // P
        # bucket per block, split by src-half
        lists = []
        for b in range(NBLK):
            m = blk == b
            sb, db = s_l[m], d_l[m]
            a_m = sb < TA_ROWS
            lists.append((sb[a_m], db[a_m], sb[~a_m] - TA_ROWS, db[~a_m]))
        per_core.append(lists)

    SA = max(
        1, max((l[0].size + P - 1) // P for lists in per_core for l in lists)
    )
    SB = max(
        1, max((l[2].size + P - 1) // P for lists in per_core for l in lists)
    )
    S = SA + SB

    in_maps = []
    for c in range(CORES):
        srcA = np.zeros((NBLK, P, SA * 8), dtype=np.int16)
        srcB = np.zeros((NBLK, P, SB * 8), dtype=np.int16)
        dstL = np.zeros((NBLK, P, S * 8), dtype=np.int16)
        drel = np.zeros((NBLK, P, S), dtype=np.uint16)
        for b in range(NBLK):
            sa, da, sb_, db = per_core[c][b]
            nA, nB = sa.size, sb_.size
            a_idx = np.zeros(SA * P, dtype=np.int16)
            a_idx[:nA] = sa
            b_idx = np.zeros(SB * P, dtype=np.int16)
            b_idx[:nB] = sb_
            d_all = np.full(S * P, PAD_DST, dtype=np.int16)
            d_all[:nA] = da
            d_all[SA * P : SA * P + nB] = db
            r_all = np.full(S * P, 127, dtype=np.float32)
            r_all[:nA] = da % P
            r_all[SA * P : SA * P + nB] = db % P
            srcA[b] = _wrap_idx(a_idx, SA * P)
            srcB[b] = _wrap_idx(b_idx, SB * P)
            dstL[b] = _wrap_idx(d_all, S * P)
            drel[b] = _f32_to_bf16_bits(r_all.reshape(S, P).T)
        m = {
            "xT": np.roll(xT, -c * NLOC, axis=1),
            "W_ext": W_ext,
            "W2": W2,
            "iota": _f32_to_bf16_bits(
                np.broadcast_to(np.arange(P, dtype=np.float32), (P, P))
            ),
            "srcA": srcA,
            "srcB": srcB,
            "dstL": dstL,
            "drel": drel,
        }
        if has_bias:
            m["bias_t"] = np.broadcast_to(bias, (P, OUT_DIM)).copy()
        if has_gamma:
            m["gamma_t"] = np.broadcast_to(ln_gamma, (P, OUT_DIM)).copy()
        if has_beta:
            m["beta_t"] = np.broadcast_to(ln_beta, (P, OUT_DIM)).copy()
        if has_resb:
            m["resb_t"] = np.broadcast_to(res_b, (P, OUT_DIM)).copy()
        in_maps.append(m)

    meta = dict(
        SA=SA, SB=SB,
        has_bias=has_bias, has_gamma=has_gamma,
        has_beta=has_beta, has_resb=has_resb,
    )
    return meta, in_maps


def build_kernel(meta: dict, dbg_blk: int = -1, stage: int = 5):
    """stage: 1=phase1 only, 2=+gathers, 3=+attention math, 4=+matmul,
    5=full (finalize)."""
    SA, SB = meta["SA"], meta["SB"]
    S = SA + SB

    nc = bacc.Bacc("TRN2", target_bir_lowering=False, debug=False, num_devices=CORES)

    xT_d = nc.dram_tensor("xT", [IN_DIM, NPAD], FP, kind="ExternalInput")
    Wext_d = nc.dram_tensor("W_ext", [IN_DIM, 264], FP, kind="ExternalInput")
    W2_d = nc.dram_tensor("W2", [IN_DIM, 264], FP, kind="ExternalInput")
    iota_d = nc.dram_tensor("iota", [P, P], BF, kind="ExternalInput")
    srcA_d = nc.dram_tensor("srcA", [NBLK, P, SA * 8], I16, kind="ExternalInput")
    srcB_d = nc.dram_tensor("srcB", [NBLK, P, SB * 8], I16, kind="ExternalInput")
    dstL_d = nc.dram_tensor("dstL", [NBLK, P, S * 8], I16, kind="ExternalInput")
    drel_d = nc.dram_tensor("drel", [NBLK, P, S], BF, kind="ExternalInput")
    opt_in = {}
    for flag, name in [
        ("has_bias", "bias_t"), ("has_gamma", "gamma_t"),
        ("has_beta", "beta_t"), ("has_resb", "resb_t"),
    ]:
        if meta[flag]:
            opt_in[name] = nc.dram_tensor(name, [P, OUT_DIM], FP, kind="ExternalInput")

    out_d = nc.dram_tensor("out", [NLOC, OUT_DIM], FP, kind="ExternalOutput")
    dbg = {}
    if dbg_blk >= 0:
        for nm, shp, dt in [
            ("dbg_G", [P, S, ROW], BF), ("dbg_Gd", [P, S, ADCOL], BF),
            ("dbg_es", [P, S, 8], FP), ("dbg_msg", [P, S, 264], BF),
            ("dbg_A", [P, S, P], BF), ("dbg_acc", [P, 264], FP),
            ("dbg_outy", [P, OUT_DIM], FP), ("dbg_mv", [P, 2], FP),
            ("dbg_z", [P, OUT_DIM], FP),
        ]:
            dbg[nm] = nc.dram_tensor(nm, shp, dt, kind="ExternalOutput")

    tabA = nc.dram_tensor("tabA", [TA_ROWS, ROW], BF)
    tabB = nc.dram_tensor("tabB", [TB_ROWS, ROW], BF)
    adst = nc.dram_tensor("adst", [ADROWS, ADCOL], BF)
    ident = nc.dram_tensor("ident", [NLOC, OUT_DIM], FP)

    NT = NPAD // P          # 392 node tiles
    CHUNK = 16              # x tiles per DMA chunk

    with tile.TileContext(nc) as tc:
        # ---------------- phase 1: h/a_src/a_dst/residual over all nodes ---
        with tc.tile_pool(name="consts", bufs=1) as consts, \
             tc.tile_pool(name="xchunk", bufs=2) as xchunk, \
             tc.tile_pool(name="p1ps", bufs=4, space="PSUM") as p1ps, \
             tc.tile_pool(name="p1row", bufs=4) as p1row:
            Wext_t = consts.tile([IN_DIM, 264], FP)
            nc.sync.dma_start(Wext_t[:], Wext_d.ap())
            W2_t = consts.tile([IN_DIM, 264], FP)
            nc.sync.dma_start(W2_t[:], W2_d.ap())
            neg_t = consts.tile([P, ADCOL], BF)
            nc.vector.memset(neg_t[:], -1.0e4)
            nc.sync.dma_start(adst.ap()[NLOC : NLOC + P, :], neg_t[:])

            eps_t = consts.tile([P, 1], FP)
            nc.vector.memset(eps_t[:], LN_EPS)

            # replicated small param tiles (only when non-trivial)
            opt_sb = {}
            for name, d in opt_in.items():
                t_ = consts.tile([P, OUT_DIM], FP)
                nc.sync.dma_start(t_[:], d.ap())
                opt_sb[name] = t_

            iota_t = consts.tile([P, P], BF)
            nc.sync.dma_start(iota_t[:], iota_d.ap())

            for ch in range((NT + CHUNK - 1) // CHUNK):
                j0 = ch * CHUNK
                jn = min(CHUNK, NT - j0)
                xc = xchunk.tile([P, CHUNK * P], FP, tag="xc")
                nc.sync.dma_start(
                    xc[:, : jn * P], xT_d.ap()[:, j0 * P : (j0 + jn) * P]
                )
                for t in range(jn):
                    j = j0 + t
                    ps1 = p1ps.tile([P, 264], FP, tag="ps")
                    nc.tensor.matmul(
                        ps1[:], lhsT=xc[:, t * P : (t + 1) * P], rhs=Wext_t[:],
                        start=True, stop=True,
                    )
                    row = p1row.tile([P, ROW], BF, tag="row")
                    nc.scalar.copy(row[:, 0:256], ps1[:, 0:256])
                    # a_src stored as error-free bf16 hi/lo pair (~f32 precision)
                    nc.scalar.copy(row[:, 256:264], ps1[:, 256:264])
                    nc.vector.tensor_tensor(
                        out=row[:, 264:272], in0=ps1[:, 256:264],
                        in1=row[:, 256:264], op=mybir.AluOpType.subtract,
                    )
                    nc.vector.memset(row[:, 272:ROW], 0.0)
                    if j < 256:
                        nc.sync.dma_start(
                            tabA.ap()[j * P : (j + 1) * P, :], row[:]
                        )
                    else:
                        jb = j - 256
                        nc.sync.dma_start(
                            tabB.ap()[jb * P : (jb + 1) * P, :], row[:]
                        )
                    if j < NBLK:  # own nodes: a_dst + residual
                        ps2 = p1ps.tile([P, 264], FP, tag="ps")
                        nc.tensor.matmul(
                            ps2[:], lhsT=xc[:, t * P : (t + 1) * P], rhs=W2_t[:],
                            start=True, stop=True,
                        )
                        ad = p1row.tile([P, ADCOL], BF, tag="ad")
                        nc.vector.memset(ad[:, 16:ADCOL], 0.0)
                        nc.scalar.copy(ad[:, 0:8], ps2[:, 0:8])
                        nc.vector.tensor_tensor(
                            out=ad[:, 8:16], in0=ps2[:, 0:8],
                            in1=ad[:, 0:8], op=mybir.AluOpType.subtract,
                        )
                        nc.sync.dma_start(
                            adst.ap()[j * P : (j + 1) * P, :], ad[:]
                        )
                        idt = p1row.tile([P, OUT_DIM], FP, tag="idt")
                        if meta["has_resb"]:
                            nc.vector.tensor_add(
                                idt[:], ps2[:, 8:264], opt_sb["resb_t"][:]
                            )
                        else:
                            nc.scalar.copy(idt[:], ps2[:, 8:264])
                        nc.sync.dma_start(
                            ident.ap()[j * P : (j + 1) * P, :], idt[:]
                        )

            tc.strict_bb_all_engine_barrier()

            # ------------- phase 2: gather + attention + aggregate ---------
            with tc.tile_pool(name="idx", bufs=3) as idxp, \
                 tc.tile_pool(name="gath", bufs=2) as gath, \
                 tc.tile_pool(name="work", bufs=2) as work, \
                 tc.tile_pool(name="aggps", bufs=2, space="PSUM") as aggps, \
                 tc.tile_pool(name="fin", bufs=3) as finp:
                for b in range(NBLK):
                    ia = idxp.tile([P, SA * 8], I16, tag="ia")
                    nc.sync.dma_start(ia[:], srcA_d.ap()[b])
                    ib = idxp.tile([P, SB * 8], I16, tag="ib")
                    nc.sync.dma_start(ib[:], srcB_d.ap()[b])
                    idl = idxp.tile([P, S * 8], I16, tag="idl")
                    nc.sync.dma_start(idl[:], dstL_d.ap()[b])
                    rel = idxp.tile([P, S], BF, tag="rel")
                    nc.sync.dma_start(rel[:], drel_d.ap()[b])

                    if stage < 2:
                        # stage 1: bypass gather/aggregate; out <- ident rows
                        fin0 = finp.tile([P, OUT_DIM], FP, tag="fin")
                        nc.sync.dma_start(
                            fin0[:], ident.ap()[b * P : (b + 1) * P, :]
                        )
                        nc.sync.dma_start(
                            out_d.ap()[b * P : (b + 1) * P, :], fin0[:]
                        )
                        continue

                    # single_packet only allows <=1024 idxs (64-desc packets)
                    G = gath.tile([P, S, ROW], BF, tag="G")
                    nc.gpsimd.dma_gather(
                        G[:, 0:SA, :], tabA.ap(), ia[:], SA * P, SA * P, ROW,
                        single_packet=SA * P <= 1024,
                    )
                    nc.gpsimd.dma_gather(
                        G[:, SA:S, :], tabB.ap(), ib[:], SB * P, SB * P, ROW,
                        single_packet=SB * P <= 1024,
                    )
                    Gd = gath.tile([P, S, ADCOL], BF, tag="Gd")
                    nc.gpsimd.dma_gather(
                        Gd[:], adst.ap(), idl[:], S * P, S * P, ADCOL,
                        single_packet=False,
                    )
                    if stage < 3:
                        # stage 2: consume gathers trivially
                        fin0 = finp.tile([P, OUT_DIM], FP, tag="fin")
                        nc.scalar.copy(fin0[:], G[:, 0, 0:256])
                        nc.vector.tensor_add(
                            fin0[:, 0:ADCOL], fin0[:, 0:ADCOL], Gd[:, 0, :]
                        )
                        nc.sync.dma_start(
                            out_d.ap()[b * P : (b + 1) * P, :], fin0[:]
                        )
                        continue

                    # logits e = a_src[src] + a_dst[dst]; w = exp(lrelu(e))
                    # e = a_src + a_dst, each a bf16 hi/lo pair
                    es = work.tile([P, S, 8], FP, tag="es")
                    nc.vector.tensor_add(es[:], G[:, :, 256:264], Gd[:, :, 0:8])
                    es2 = work.tile([P, S, 8], FP, tag="es2")
                    nc.vector.tensor_add(es2[:], G[:, :, 264:272], Gd[:, :, 8:16])
                    nc.vector.tensor_add(es[:], es[:], es2[:])
                    # leaky relu = max(x, 0.2 x)
                    lr = work.tile([P, S, 8], FP, tag="lr")
                    nc.vector.tensor_scalar_mul(lr[:], es[:], NEG_SLOPE)
                    nc.vector.tensor_tensor(
                        out=lr[:], in0=es[:], in1=lr[:], op=mybir.AluOpType.max
                    )
                    msg = work.tile([P, S, 264], BF, tag="msg")
                    nc.scalar.activation(
                        msg[:, :, 256:264], lr[:], mybir.ActivationFunctionType.Exp
                    )
                    # w replicated x32 (ACT copy with 0-step AP), then msg mul
                    wrep = work.tile([P, S, 256], BF, tag="wrep")
                    wsrc = msg[:, :, 256:264]
                    nc.scalar.copy(
                        wrep[:],
                        bass.AP(tensor=wsrc.tensor, offset=wsrc.offset,
                                ap=[wsrc.ap[0], [264, S], [1, 8], [0, 32]]),
                    )
                    nc.vector.tensor_mul(msg[:, :, 0:256], G[:, :, 0:256], wrep[:])

                    # one-hot A[p, t, j] = (drel[p, t] == iota[p, j])
                    A = work.tile([P, S, P], BF, tag="A")
                    nc.vector.tensor_tensor(
                        out=A[:],
                        in0=bass.AP(tensor=iota_t.tensor, offset=iota_t[:].offset,
                                    ap=[iota_t[:].ap[0], [0, S], [1, P]]),
                        in1=bass.AP(tensor=rel.tensor, offset=rel[:].offset,
                                    ap=[rel[:].ap[0], [1, S], [0, P]]),
                        op=mybir.AluOpType.is_equal,
                    )

                    if stage < 4:
                        # stage 3: consume attention math trivially
                        fin0 = finp.tile([P, OUT_DIM], FP, tag="fin")
                        nc.scalar.copy(fin0[:], msg[:, 0, 0:256])
                        nc.vector.tensor_add(
                            fin0[:, 0:P], fin0[:, 0:P], A[:, 0, :]
                        )
                        nc.sync.dma_start(
                            out_d.ap()[b * P : (b + 1) * P, :], fin0[:]
                        )
                        continue

                    acc = aggps.tile([P, 264], FP, tag="acc")
                    for t in range(S):
                        nc.tensor.matmul(
                            acc[:], lhsT=A[:, t, :], rhs=msg[:, t, :],
                            start=(t == 0), stop=(t == S - 1),
                        )
                    if stage < 5:
                        # stage 4: dump accumulators, skip finalize
                        fin0 = finp.tile([P, OUT_DIM], FP, tag="fin")
                        nc.scalar.copy(fin0[:], acc[:, 0:256])
                        nc.sync.dma_start(
                            out_d.ap()[b * P : (b + 1) * P, :], fin0[:]
                        )
                        continue

                    if b == dbg_blk:
                        nc.sync.dma_start(dbg["dbg_G"].ap(), G[:])
                        nc.sync.dma_start(dbg["dbg_Gd"].ap(), Gd[:])
                        nc.sync.dma_start(dbg["dbg_es"].ap(), es[:])
                        nc.sync.dma_start(dbg["dbg_msg"].ap(), msg[:])
                        nc.sync.dma_start(dbg["dbg_A"].ap(), A[:])
                        acc_sb = finp.tile([P, 264], FP, tag="accsb")
                        nc.scalar.copy(acc_sb[:], acc[:])
                        nc.sync.dma_start(dbg["dbg_acc"].ap(), acc_sb[:])

                    # ---- finalize block: divide, LN, residual, ELU ----
                    rec = finp.tile([P, 8], FP, tag="rec")
                    nc.vector.reciprocal(rec[:], acc[:, 256:264])
                    outy = finp.tile([P, OUT_DIM], FP, tag="outy")
                    accv = acc[:, 0:256]
                    nc.vector.tensor_mul(
                        outy[:],
                        bass.AP(tensor=accv.tensor, offset=accv.offset,
                                ap=[accv.ap[0], [32, 8], [1, 32]]),
                        bass.AP(tensor=rec.tensor, offset=rec[:].offset,
                                ap=[rec[:].ap[0], [1, 8], [0, 32]]),
                    )
                    if meta["has_bias"]:
                        nc.vector.tensor_add(outy[:], outy[:], opt_sb["bias_t"][:])
                    stats = finp.tile([P, 6], FP, tag="stats")
                    nc.vector.bn_stats(stats[:], outy[:])
                    mv = finp.tile([P, 2], FP, tag="mv")
                    nc.vector.bn_aggr(mv[:], stats[:])
                    std = finp.tile([P, 1], FP, tag="std")
                    nc.scalar.activation(
                        std[:], mv[:, 1:2], mybir.ActivationFunctionType.Sqrt,
                        bias=eps_t[:],
                    )
                    rstd = finp.tile([P, 1], FP, tag="rstd")
                    nc.vector.reciprocal(rstd[:], std[:])
                    z = finp.tile([P, OUT_DIM], FP, tag="z")
                    nc.vector.tensor_scalar(
                        out=z[:], in0=outy[:],
                        scalar1=mv[:, 0:1], scalar2=rstd[:],
                        op0=mybir.AluOpType.subtract, op1=mybir.AluOpType.mult,
                    )
                    if meta["has_gamma"]:
                        nc.vector.tensor_mul(z[:], z[:], opt_sb["gamma_t"][:])
                    if meta["has_beta"]:
                        nc.vector.tensor_add(z[:], z[:], opt_sb["beta_t"][:])
                    idt = finp.tile([P, OUT_DIM], FP, tag="idt2")
                    nc.sync.dma_start(idt[:], ident.ap()[b * P : (b + 1) * P, :])
                    nc.vector.tensor_add(z[:], z[:], idt[:])
                    if b == dbg_blk:
                        nc.sync.dma_start(dbg["dbg_outy"].ap(), outy[:])
                        nc.sync.dma_start(dbg["dbg_mv"].ap(), mv[:])
                        nc.sync.dma_start(dbg["dbg_z"].ap(), z[:])
                    # ELU(z) = max(z, exp(min(z, 0)) - 1)
                    zm = finp.tile([P, OUT_DIM], FP, tag="zm")
                    nc.vector.tensor_scalar_min(zm[:], z[:], 0.0)
                    ez = finp.tile([P, OUT_DIM], FP, tag="ez")
                    nc.scalar.activation(
                        ez[:], zm[:], mybir.ActivationFunctionType.Exp
                    )
                    nc.vector.tensor_scalar_add(ez[:], ez[:], -1.0)
                    fin = finp.tile([P, OUT_DIM], FP, tag="fin")
                    nc.vector.tensor_tensor(
                        out=fin[:], in0=z[:], in1=ez[:], op=mybir.AluOpType.max
                    )
                    nc.sync.dma_start(out_d.ap()[b * P : (b + 1) * P, :], fin[:])

    nc.compile()
    return nc


_CACHE = {}


def kernel(**inputs) -> np.ndarray:
    meta, in_maps = prepare_host(inputs)
    key = tuple(sorted(meta.items()))
    if key not in _CACHE:
        _CACHE[key] = build_kernel(meta)
    nc = _CACHE[key]
    res = run_bass_kernel_spmd(nc, in_maps, list(range(CORES)))
    full = np.concatenate([res.results[c]["out"] for c in range(CORES)], axis=0)
    return np.ascontiguousarray(full[:N])


# revision 31
# speedup vs baseline: 1.4107x; 1.4107x over previous
"""GAT layer (GATConv + LayerNorm + residual + ELU) as a distributed Bass kernel
on 8 Trainium2 NeuronCores.

Distribution strategy (graph/data parallel, per sharding hint):
  - Nodes are partitioned across the 8 cores (6272 per core); inputs are
    rotated per-core so each core's own nodes are local rows 0..6271. Small
    params (W, att, ln, res) are replicated; each core computes the full
    h-table (replicated matmul) into its own DRAM — the "halo gather" of
    remote source features is then a core-local dma_gather per edge.
  - Edges are assigned to the core owning their destination and sorted by
    destination, so segment softmax + scatter-add stay core-local.

Algorithm notes:
  - att_src/att_dst dot products fold into extra columns of W, so a_src and
    a_dst come out of the same matmul that produces h (one pass over x).
  - Segment softmax: max-subtraction is skipped (logits are O(10); exp cannot
    overflow in f32; mathematically identical) and the per-edge normalization
    folds out: out[n] = sum_e w_e h_src / sum_e w_e. Numerator + denominator
    accumulate in a single PSUM matmul by appending the w columns to the
    message.
  - Scatter-add = TensorE matmul with per-128-dst-block one-hot matrices.
    The edge->dst-block assignment is host-known, so the one-hots (A) and
    their transposes (AT, used to expand a_dst[dst] to edges) are precomputed
    on the host and streamed in as bf16 {0,1} matrices. Padding slots have
    all-zero one-hot rows, which also makes gather padding harmless.
  - Self-loop edges never enter the edge lists: each dst block processes one
    extra "self" subtile whose rows come from a contiguous table read and
    whose one-hot is the identity.
  - a_src/a_dst are stored as error-free bf16 hi+lo pairs (~f32 precision).
  - LayerNorm + residual + ELU run as a second, batched pass (7 blocks per
    chunk) so the ACT engine's exp<->sqrt table reloads happen per chunk, not
    per block.
"""
import sys

sys.path.insert(0, "/opt/trn_rl_repo")

import numpy as np

import concourse.bass as bass
import concourse.bacc as bacc
import concourse.tile as tile
from concourse import mybir
from concourse.bass_utils import run_bass_kernel_spmd

# ---- problem constants (hardcoded per contract) ----
N = 50000
E = 800000
IN_DIM = 128
OUT_DIM = 256
HEADS = 8
NEG_SLOPE = 0.2
LN_EPS = 1e-5

CORES = 8
P = 128
NLOC = 6272                   # own nodes per core (49 blocks of 128)
NPAD = NLOC * CORES           # 50176 padded node count
NBLK = NLOC // P              # 49 dst blocks per core
CHUNK_BLKS = 7                # finalize batching (49 = 7 x 7)
TA_ROWS = 32768               # gather table A rows (int16 index limit)
TB_ROWS = NPAD - TA_ROWS      # 17408
ROW = 384                     # bf16/row: [h:256 | a_src hi:8 | lo:8 | junk pad]
RWR = 272                     # columns actually written per row
ADW = 16                      # bf16 per a_dst row: [hi:8 | lo:8]

FP = mybir.dt.float32
BF = mybir.dt.bfloat16
I16 = mybir.dt.int16

ONE_BF16 = np.uint16(0x3F80)


def _wrap_idx(vals: np.ndarray) -> np.ndarray:
    """dma_gather index layout: slot i -> [i % 16, i // 16], replicated to
    128 partitions."""
    n = vals.size
    assert n % 16 == 0
    w = np.zeros((16, n // 16), dtype=np.int16)
    w[np.arange(n) % 16, np.arange(n) // 16] = vals.astype(np.int16)
    return np.tile(w, (8, 1))


def prepare_host(inputs: dict) -> tuple[dict, list[dict]]:
    x = np.asarray(inputs["x"], dtype=np.float32)
    edge_index = np.asarray(inputs["edge_index"])
    W = np.asarray(inputs["W"], dtype=np.float32)
    att_src = np.asarray(inputs["att_src"], dtype=np.float32)
    att_dst = np.asarray(inputs["att_dst"], dtype=np.float32)
    bias = np.asarray(inputs["bias"], dtype=np.float32)
    ln_gamma = np.asarray(inputs["ln_gamma"], dtype=np.float32)
    ln_beta = np.asarray(inputs["ln_beta"], dtype=np.float32)
    res_W = np.asarray(inputs["res_W"], dtype=np.float32)
    res_b = np.asarray(inputs["res_b"], dtype=np.float32)

    W3 = W.reshape(IN_DIM, HEADS, OUT_DIM // HEADS)
    Wsrc = np.einsum("ihc,hc->ih", W3, att_src)
    Wdst = np.einsum("ihc,hc->ih", W3, att_dst)
    W_ext = np.concatenate([W, Wsrc], axis=1)          # [128, 264]
    W2 = np.concatenate([Wdst, res_W.T], axis=1)       # [128, 264]

    meta = dict(
        has_bias=bool(np.any(bias != 0.0)),
        has_gamma=bool(np.any(ln_gamma != 1.0)),
        has_beta=bool(np.any(ln_beta != 0.0)),
        has_resb=bool(np.any(res_b != 0.0)),
    )

    xT = np.zeros((IN_DIM, NPAD), dtype=np.float32)
    xT[:, :N] = x.T

    src = edge_index[0].astype(np.int64)
    dst = edge_index[1].astype(np.int64)
    owner = dst // NLOC

    per_core = []
    for c in range(CORES):
        sel = owner == c
        s_l = (src[sel] - c * NLOC) % NPAD
        d_l = dst[sel] - c * NLOC
        order = np.argsort(d_l, kind="stable")
        s_l, d_l = s_l[order], d_l[order]
        blk = d_l // P
        lists = []
        for b in range(NBLK):
            m = blk == b
            sb_, db_ = s_l[m], d_l[m]
            a_m = sb_ < TA_ROWS
            lists.append((sb_[a_m], db_[a_m] % P, sb_[~a_m] - TA_ROWS,
                          db_[~a_m] % P))
        per_core.append(lists)

    SA = max(1, max((l[0].size + P - 1) // P for ls in per_core for l in ls))
    SB = max(1, max((l[2].size + P - 1) // P for ls in per_core for l in ls))
    S = SA + SB
    meta.update(SA=SA, SB=SB)

    jj = np.arange(P)
    in_maps = []
    for c in range(CORES):
        srcA = np.zeros((NBLK, P, SA * 8), dtype=np.int16)
        srcB = np.zeros((NBLK, P, SB * 8), dtype=np.int16)
        At = np.zeros((NBLK, P, S * P), dtype=np.uint16)
        AT = np.zeros((NBLK, P, S * P), dtype=np.uint16)
        for b in range(NBLK):
            sa, ra, sb_, rb = per_core[c][b]
            nA, nB = sa.size, sb_.size
            a_idx = np.zeros(SA * P, dtype=np.int16)
            a_idx[:nA] = sa
            b_idx = np.zeros(SB * P, dtype=np.int16)
            b_idx[:nB] = sb_
            rel = np.full(S * P, -1, dtype=np.int64)
            rel[:nA] = ra
            rel[SA * P : SA * P + nB] = rb
            rel3 = rel.reshape(S, P)
            # A [p=edge, (t, j)] ; AT [p=j, (t, e)]
            at = (rel3[:, :, None] == jj[None, None, :])      # [t, e, j]
            At[b] = np.where(at.transpose(1, 0, 2).reshape(P, S * P),
                             ONE_BF16, 0)
            AT[b] = np.where(at.transpose(2, 0, 1).reshape(P, S * P),
                             ONE_BF16, 0)
            srcA[b] = _wrap_idx(a_idx)
            srcB[b] = _wrap_idx(b_idx)
        m = {
            "xT": np.roll(xT, -c * NLOC, axis=1),
            "W_ext": W_ext,
            "W2": W2,
            "ident128": np.where(np.eye(P, dtype=bool), ONE_BF16, 0).astype(
                np.uint16
            ),
            "srcA": srcA,
            "srcB": srcB,
            "At": At,
            "AT": AT,
        }
        if meta["has_bias"]:
            m["bias_t"] = np.broadcast_to(bias, (P, OUT_DIM)).copy()
        if meta["has_gamma"]:
            m["gamma_t"] = np.broadcast_to(ln_gamma, (P, OUT_DIM)).copy()
        if meta["has_beta"]:
            m["beta_t"] = np.broadcast_to(ln_beta, (P, OUT_DIM)).copy()
        if meta["has_resb"]:
            m["resb_t"] = np.broadcast_to(res_b, (P, OUT_DIM)).copy()
        in_maps.append(m)

    return meta, in_maps


def build_kernel(meta: dict):
    SA, SB = meta["SA"], meta["SB"]
    S = SA + SB
    SS = S + 1          # +1 self subtile

    nc = bacc.Bacc("TRN2", target_bir_lowering=False, debug=False,
                   num_devices=CORES)

    xT_d = nc.dram_tensor("xT", [IN_DIM, NPAD], FP, kind="ExternalInput")
    Wext_d = nc.dram_tensor("W_ext", [IN_DIM, 264], FP, kind="ExternalInput")
    W2_d = nc.dram_tensor("W2", [IN_DIM, 264], FP, kind="ExternalInput")
    id_d = nc.dram_tensor("ident128", [P, P], BF, kind="ExternalInput")
    srcA_d = nc.dram_tensor("srcA", [NBLK, P, SA * 8], I16, kind="ExternalInput")
    srcB_d = nc.dram_tensor("srcB", [NBLK, P, SB * 8], I16, kind="ExternalInput")
    At_d = nc.dram_tensor("At", [NBLK, P, S * P], BF, kind="ExternalInput")
    AT_d = nc.dram_tensor("AT", [NBLK, P, S * P], BF, kind="ExternalInput")
    opt_in = {}
    for flag, name in [
        ("has_bias", "bias_t"), ("has_gamma", "gamma_t"),
        ("has_beta", "beta_t"), ("has_resb", "resb_t"),
    ]:
        if meta[flag]:
            opt_in[name] = nc.dram_tensor(name, [P, OUT_DIM], FP,
                                          kind="ExternalInput")

    out_d = nc.dram_tensor("out", [NLOC, OUT_DIM], FP, kind="ExternalOutput")

    tabA = nc.dram_tensor("tabA", [TA_ROWS, ROW], BF)
    tabB = nc.dram_tensor("tabB", [TB_ROWS, ROW], BF)
    adst = nc.dram_tensor("adst", [NLOC, ADW], BF)
    ident = nc.dram_tensor("ident", [NLOC, OUT_DIM], FP)
    stage = [
        nc.dram_tensor(f"stage{cch}", [CHUNK_BLKS * P, OUT_DIM], FP)
        for cch in range(NBLK // CHUNK_BLKS)
    ]

    NT = NPAD // P
    XCH = 16

    with tile.TileContext(nc) as tc:
        with tc.tile_pool(name="consts", bufs=1) as consts, \
             tc.tile_pool(name="xchunk", bufs=2) as xchunk, \
             tc.tile_pool(name="p1ps", bufs=4, space="PSUM") as p1ps, \
             tc.tile_pool(name="p1row", bufs=4) as p1row:
            Wext_t = consts.tile([IN_DIM, 264], FP)
            nc.sync.dma_start(Wext_t[:], Wext_d.ap())
            W2_t = consts.tile([IN_DIM, 264], FP)
            nc.sync.dma_start(W2_t[:], W2_d.ap())
            id_t = consts.tile([P, P], BF)
            nc.sync.dma_start(id_t[:], id_d.ap())
            eps_t = consts.tile([P, 1], FP)
            nc.vector.memset(eps_t[:], LN_EPS)
            opt_sb = {}
            for name, dd in opt_in.items():
                t_ = consts.tile([P, OUT_DIM], FP)
                nc.sync.dma_start(t_[:], dd.ap())
                opt_sb[name] = t_

            # ---------------- phase 1: h | a_src | a_dst | residual --------
            for ch in range((NT + XCH - 1) // XCH):
                j0 = ch * XCH
                jn = min(XCH, NT - j0)
                xc = xchunk.tile([P, XCH * P], FP, tag="xc")
                nc.sync.dma_start(
                    xc[:, : jn * P], xT_d.ap()[:, j0 * P : (j0 + jn) * P]
                )
                for t in range(jn):
                    j = j0 + t
                    ps1 = p1ps.tile([P, 264], FP, tag="ps")
                    nc.tensor.matmul(
                        ps1[:], lhsT=xc[:, t * P : (t + 1) * P], rhs=Wext_t[:],
                        start=True, stop=True,
                    )
                    row = p1row.tile([P, RWR], BF, tag="row")
                    nc.scalar.copy(row[:, 0:256], ps1[:, 0:256])
                    nc.scalar.copy(row[:, 256:264], ps1[:, 256:264])
                    nc.vector.tensor_tensor(
                        out=row[:, 264:272], in0=ps1[:, 256:264],
                        in1=row[:, 256:264], op=mybir.AluOpType.subtract,
                    )
                    # rows are 384 wide; only 272 written (tail never read)
                    if j < 256:
                        dst_rows = tabA.ap()[j * P : (j + 1) * P, 0:RWR]
                    else:
                        jb = j - 256
                        dst_rows = tabB.ap()[jb * P : (jb + 1) * P, 0:RWR]
                    nc.sync.dma_start(dst_rows, row[:])
                    if j < NBLK:
                        ps2 = p1ps.tile([P, 264], FP, tag="ps")
                        nc.tensor.matmul(
                            ps2[:], lhsT=xc[:, t * P : (t + 1) * P],
                            rhs=W2_t[:], start=True, stop=True,
                        )
                        ad = p1row.tile([P, ADW], BF, tag="ad")
                        nc.scalar.copy(ad[:, 0:8], ps2[:, 0:8])
                        nc.vector.tensor_tensor(
                            out=ad[:, 8:16], in0=ps2[:, 0:8],
                            in1=ad[:, 0:8], op=mybir.AluOpType.subtract,
                        )
                        nc.sync.dma_start(
                            adst.ap()[j * P : (j + 1) * P, :], ad[:]
                        )
                        idt = p1row.tile([P, OUT_DIM], FP, tag="idt")
                        if meta["has_resb"]:
                            nc.vector.tensor_add(
                                idt[:], ps2[:, 8:264], opt_sb["resb_t"][:]
                            )
                        else:
                            nc.scalar.copy(idt[:], ps2[:, 8:264])
                        nc.sync.dma_start(
                            ident.ap()[j * P : (j + 1) * P, :], idt[:]
                        )

            tc.strict_bb_all_engine_barrier()

            # ------------- phase 2: gather + attention + aggregate ---------
            with tc.tile_pool(name="idx", bufs=3) as idxp, \
                 tc.tile_pool(name="hostA", bufs=2) as hostA, \
                 tc.tile_pool(name="gath", bufs=2) as gath, \
                 tc.tile_pool(name="work", bufs=2) as work, \
                 tc.tile_pool(name="adps", bufs=2, space="PSUM") as adps, \
                 tc.tile_pool(name="aggps", bufs=2, space="PSUM") as aggps, \
                 tc.tile_pool(name="fin", bufs=3) as finp, \
                 tc.tile_pool(name="fin2", bufs=1) as finp2:
                for b in range(NBLK):
                    ia = idxp.tile([P, SA * 8], I16, tag="ia")
                    nc.sync.dma_start(ia[:], srcA_d.ap()[b])
                    ib = idxp.tile([P, SB * 8], I16, tag="ib")
                    nc.sync.dma_start(ib[:], srcB_d.ap()[b])
                    At = hostA.tile([P, S * P], BF, tag="At")
                    nc.sync.dma_start(At[:], At_d.ap()[b])
                    AT = hostA.tile([P, S * P], BF, tag="AT")
                    nc.sync.dma_start(AT[:], AT_d.ap()[b])
                    adblk = idxp.tile([P, ADW], BF, tag="adblk")
                    nc.sync.dma_start(
                        adblk[:], adst.ap()[b * P : (b + 1) * P, :]
                    )

                    G = gath.tile([P, SS, ROW], BF, tag="G")
                    nc.gpsimd.dma_gather(
                        G[:, 0:SA, :], tabA.ap(), ia[:], SA * P, SA * P, ROW,
                        single_packet=SA * P <= 1024,
                    )
                    nc.gpsimd.dma_gather(
                        G[:, SA:S, :], tabB.ap(), ib[:], SB * P, SB * P, ROW,
                        single_packet=SB * P <= 1024,
                    )
                    # self subtile: contiguous table rows of this block
                    nc.sync.dma_start(
                        G[:, S, 0:RWR],
                        tabA.ap()[b * P : (b + 1) * P, 0:RWR],
                    )

                    # a_dst expanded dst->edge via AT one-hot matmuls
                    # (each matmul gets its own PSUM tile; copy into SBUF)
                    pad = work.tile([P, S, ADW], FP, tag="pad")
                    for t in range(S):
                        pps = adps.tile([P, ADW], FP, tag="pps")
                        nc.tensor.matmul(
                            pps[:], lhsT=AT[:, t * P : (t + 1) * P],
                            rhs=adblk[:], start=True, stop=True,
                        )
                        nc.scalar.copy(pad[:, t, :], pps[:])

                    # logits: hi+lo pairs summed
                    es = work.tile([P, SS, 8], FP, tag="es")
                    nc.vector.tensor_add(
                        es[:, 0:S, :], G[:, 0:S, 256:264], G[:, 0:S, 264:272]
                    )
                    es2 = work.tile([P, S, 8], FP, tag="es2")
                    nc.vector.tensor_add(
                        es2[:], pad[:, :, 0:8], pad[:, :, 8:16]
                    )
                    nc.vector.tensor_add(es[:, 0:S, :], es[:, 0:S, :], es2[:])
                    # self logits
                    sads = work.tile([P, 8], FP, tag="sads")
                    nc.vector.tensor_add(
                        sads[:], adblk[:, 0:8], adblk[:, 8:16]
                    )
                    nc.vector.tensor_add(
                        es[:, S, :], G[:, S, 256:264], G[:, S, 264:272]
                    )
                    nc.vector.tensor_add(es[:, S, :], es[:, S, :], sads[:])

                    # w = exp(max(e, 0.2 e)) -> msg[:, :, 256:264] (bf16)
                    lr = work.tile([P, SS, 8], FP, tag="lr")
                    nc.vector.tensor_scalar_mul(lr[:], es[:], NEG_SLOPE)
                    nc.vector.tensor_tensor(
                        out=lr[:], in0=es[:], in1=lr[:], op=mybir.AluOpType.max
                    )
                    msg = work.tile([P, SS, 264], BF, tag="msg")
                    nc.scalar.activation(
                        msg[:, :, 256:264], lr[:],
                        mybir.ActivationFunctionType.Exp,
                    )
                    wrep = work.tile([P, SS, 256], BF, tag="wrep")
                    wsrc = msg[:, :, 256:264]
                    nc.scalar.copy(
                        wrep[:],
                        bass.AP(tensor=wsrc.tensor, offset=wsrc.offset,
                                ap=[wsrc.ap[0], [264, SS], [1, 8], [0, 32]]),
                    )
                    nc.vector.tensor_mul(
                        msg[:, :, 0:256], G[:, :, 0:256], wrep[:]
                    )

                    acc = aggps.tile([P, 264], FP, tag="acc")
                    for t in range(SS):
                        lhsT = id_t[:] if t == S else At[:, t * P : (t + 1) * P]
                        nc.tensor.matmul(
                            acc[:], lhsT=lhsT, rhs=msg[:, t, :],
                            start=(t == 0), stop=(t == SS - 1),
                        )

                    # pass 1 finalize: normalize, stash
                    rec = finp.tile([P, 8], FP, tag="rec")
                    nc.vector.reciprocal(rec[:], acc[:, 256:264])
                    outy = finp.tile([P, OUT_DIM], FP, tag="outy")
                    accv = acc[:, 0:256]
                    nc.vector.tensor_mul(
                        outy[:],
                        bass.AP(tensor=accv.tensor, offset=accv.offset,
                                ap=[accv.ap[0], [32, 8], [1, 32]]),
                        bass.AP(tensor=rec.tensor, offset=rec[:].offset,
                                ap=[rec[:].ap[0], [1, 8], [0, 32]]),
                    )
                    if meta["has_bias"]:
                        nc.vector.tensor_add(
                            outy[:], outy[:], opt_sb["bias_t"][:]
                        )
                    cch, kk = divmod(b, CHUNK_BLKS)
                    nc.sync.dma_start(
                        stage[cch].ap()[kk * P : (kk + 1) * P, :], outy[:]
                    )

                    if kk != CHUNK_BLKS - 1:
                        continue

                    # ---- pass 2 (batched finalize for chunk cch) ----
                    K = CHUNK_BLKS
                    oy = finp2.tile([P, K, OUT_DIM], FP, tag="oy")
                    for k in range(K):
                        nc.sync.dma_start(
                            oy[:, k, :],
                            stage[cch].ap()[k * P : (k + 1) * P, :],
                        )
                    # mean / var per block (manual, batched over K blocks)
                    mean = finp2.tile([P, K], FP, tag="mean")
                    nc.vector.tensor_reduce(
                        mean[:], oy[:], axis=mybir.AxisListType.X,
                        op=mybir.AluOpType.add,
                    )
                    nc.vector.tensor_scalar_mul(mean[:], mean[:], 1.0 / OUT_DIM)
                    sq = finp2.tile([P, K, OUT_DIM], FP, tag="sq")
                    nc.vector.tensor_mul(sq[:], oy[:], oy[:])
                    var = finp2.tile([P, K], FP, tag="var")
                    nc.vector.tensor_reduce(
                        var[:], sq[:], axis=mybir.AxisListType.X,
                        op=mybir.AluOpType.add,
                    )
                    nc.vector.tensor_scalar_mul(var[:], var[:], 1.0 / OUT_DIM)
                    msq = finp2.tile([P, K], FP, tag="msq")
                    nc.vector.tensor_mul(msq[:], mean[:], mean[:])
                    nc.vector.tensor_tensor(
                        out=var[:], in0=var[:], in1=msq[:],
                        op=mybir.AluOpType.subtract,
                    )
                    std = finp2.tile([P, K], FP, tag="std")
                    nc.scalar.activation(
                        std[:], var[:],
                        mybir.ActivationFunctionType.Sqrt, bias=eps_t[:],
                    )
                    rstd = finp2.tile([P, K], FP, tag="rstd")
                    nc.vector.reciprocal(rstd[:], std[:])
                    z = finp2.tile([P, K, OUT_DIM], FP, tag="z")
                    for k in range(K):
                        nc.vector.tensor_scalar(
                            out=z[:, k, :], in0=oy[:, k, :],
                            scalar1=mean[:, k : k + 1],
                            scalar2=rstd[:, k : k + 1],
                            op0=mybir.AluOpType.subtract,
                            op1=mybir.AluOpType.mult,
                        )
                        if meta["has_gamma"]:
                            nc.vector.tensor_mul(
                                z[:, k, :], z[:, k, :], opt_sb["gamma_t"][:]
                            )
                        if meta["has_beta"]:
                            nc.vector.tensor_add(
                                z[:, k, :], z[:, k, :], opt_sb["beta_t"][:]
                            )
                    idt = finp2.tile([P, K, OUT_DIM], FP, tag="idt2")
                    for k in range(K):
                        nc.sync.dma_start(
                            idt[:, k, :],
                            ident.ap()[(cch * K + k) * P : (cch * K + k + 1) * P, :],
                        )
                    nc.vector.tensor_add(z[:], z[:], idt[:])
                    # ELU(z) = max(z, exp(min(z, 0)) - 1)
                    zm = finp2.tile([P, K, OUT_DIM], FP, tag="zm")
                    nc.vector.tensor_scalar_min(zm[:], z[:], 0.0)
                    ez = finp2.tile([P, K, OUT_DIM], FP, tag="ez")
                    nc.scalar.activation(
                        ez[:], zm[:], mybir.ActivationFunctionType.Exp
                    )
                    nc.vector.tensor_scalar_add(ez[:], ez[:], -1.0)
                    fin = finp2.tile([P, K, OUT_DIM], FP, tag="fin")
                    nc.vector.tensor_tensor(
                        out=fin[:], in0=z[:], in1=ez[:],
                        op=mybir.AluOpType.max,
                    )
                    for k in range(K):
                        nc.sync.dma_start(
                            out_d.ap()[(cch * K + k) * P : (cch * K + k + 1) * P, :],
                            fin[:, k, :],
                        )

    nc.compile()
    return nc


_CACHE = {}


def kernel(**inputs) -> np.ndarray:
    meta, in_maps = prepare_host(inputs)
    key = tuple(sorted(meta.items()))
    if key not in _CACHE:
        _CACHE[key] = build_kernel(meta)
    nc = _CACHE[key]
    res = run_bass_kernel_spmd(nc, in_maps, list(range(CORES)))
    full = np.concatenate([res.results[c]["out"] for c in range(CORES)], axis=0)
    return np.ascontiguousarray(full[:N])


# revision 32
# speedup vs baseline: 1.4227x; 1.0085x over previous
"""GAT layer (GATConv + LayerNorm + residual + ELU) as a distributed Bass kernel
on 8 Trainium2 NeuronCores.

Distribution strategy (graph/data parallel, per sharding hint):
  - Nodes are partitioned across the 8 cores (6272 per core); inputs are
    rotated per-core so each core's own nodes are local rows 0..6271. Small
    params (W, att, ln, res) are replicated; each core computes the full
    h-table (replicated matmul) into its own DRAM — the "halo gather" of
    remote source features is then a core-local dma_gather per edge.
  - Edges are assigned to the core owning their destination and sorted by
    destination, so segment softmax + scatter-add stay core-local.

Algorithm notes:
  - att_src/att_dst dot products fold into extra columns of W, so a_src and
    a_dst come out of the same matmul that produces h (one pass over x).
  - Segment softmax: max-subtraction is skipped (logits are O(10); exp cannot
    overflow in f32; mathematically identical) and the per-edge normalization
    folds out: out[n] = sum_e w_e h_src / sum_e w_e. Numerator + denominator
    accumulate in a single PSUM matmul by appending the w columns to the
    message.
  - Scatter-add = TensorE matmul with per-128-dst-block one-hot matrices.
    The edge->dst-block assignment is host-known, so the one-hots (A) and
    their transposes (AT, used to expand a_dst[dst] to edges) are precomputed
    on the host and streamed in as bf16 {0,1} matrices. Padding slots have
    all-zero one-hot rows, which also makes gather padding harmless.
  - Self-loop edges never enter the edge lists: each dst block processes one
    extra "self" subtile whose rows come from a contiguous table read and
    whose one-hot is the identity.
  - a_src/a_dst are stored as error-free bf16 hi+lo pairs (~f32 precision).
  - LayerNorm + residual + ELU run as a second, batched pass (7 blocks per
    chunk) so the ACT engine's exp<->sqrt table reloads happen per chunk, not
    per block.
"""
import sys

sys.path.insert(0, "/opt/trn_rl_repo")

import numpy as np

import concourse.bass as bass
import concourse.bacc as bacc
import concourse.tile as tile
from concourse import mybir
from concourse.bass_utils import run_bass_kernel_spmd

# ---- problem constants (hardcoded per contract) ----
N = 50000
E = 800000
IN_DIM = 128
OUT_DIM = 256
HEADS = 8
NEG_SLOPE = 0.2
LN_EPS = 1e-5

CORES = 8
P = 128
NLOC = 6272                   # own nodes per core (49 blocks of 128)
NPAD = NLOC * CORES           # 50176 padded node count
NBLK = NLOC // P              # 49 dst blocks per core
CHUNK_BLKS = 7                # finalize batching (49 = 7 x 7)
TA_ROWS = 32768               # gather table A rows (int16 index limit)
TB_ROWS = NPAD - TA_ROWS      # 17408
ROW = 384                     # bf16/row: [h:256 | a_src hi:8 | lo:8 | junk pad]
RWR = 272                     # columns actually written per row
ADW = 16                      # bf16 per a_dst row: [hi:8 | lo:8]

FP = mybir.dt.float32
BF = mybir.dt.bfloat16
I16 = mybir.dt.int16

ONE_BF16 = np.uint16(0x3F80)


def _wrap_idx(vals: np.ndarray) -> np.ndarray:
    """dma_gather index layout: slot i -> [i % 16, i // 16], replicated to
    128 partitions."""
    n = vals.size
    assert n % 16 == 0
    w = np.zeros((16, n // 16), dtype=np.int16)
    w[np.arange(n) % 16, np.arange(n) // 16] = vals.astype(np.int16)
    return np.tile(w, (8, 1))


def prepare_host(inputs: dict) -> tuple[dict, list[dict]]:
    x = np.asarray(inputs["x"], dtype=np.float32)
    edge_index = np.asarray(inputs["edge_index"])
    W = np.asarray(inputs["W"], dtype=np.float32)
    att_src = np.asarray(inputs["att_src"], dtype=np.float32)
    att_dst = np.asarray(inputs["att_dst"], dtype=np.float32)
    bias = np.asarray(inputs["bias"], dtype=np.float32)
    ln_gamma = np.asarray(inputs["ln_gamma"], dtype=np.float32)
    ln_beta = np.asarray(inputs["ln_beta"], dtype=np.float32)
    res_W = np.asarray(inputs["res_W"], dtype=np.float32)
    res_b = np.asarray(inputs["res_b"], dtype=np.float32)

    W3 = W.reshape(IN_DIM, HEADS, OUT_DIM // HEADS)
    Wsrc = np.einsum("ihc,hc->ih", W3, att_src)
    Wdst = np.einsum("ihc,hc->ih", W3, att_dst)
    W_ext = np.concatenate([W, Wsrc], axis=1)          # [128, 264]
    W2 = np.concatenate([Wdst, res_W.T], axis=1)       # [128, 264]

    meta = dict(
        has_bias=bool(np.any(bias != 0.0)),
        has_gamma=bool(np.any(ln_gamma != 1.0)),
        has_beta=bool(np.any(ln_beta != 0.0)),
        has_resb=bool(np.any(res_b != 0.0)),
    )

    xT = np.zeros((IN_DIM, NPAD), dtype=np.float32)
    xT[:, :N] = x.T

    src = edge_index[0].astype(np.int64)
    dst = edge_index[1].astype(np.int64)
    owner = dst // NLOC

    per_core = []
    for c in range(CORES):
        sel = owner == c
        s_l = (src[sel] - c * NLOC) % NPAD
        d_l = dst[sel] - c * NLOC
        order = np.argsort(d_l, kind="stable")
        s_l, d_l = s_l[order], d_l[order]
        blk = d_l // P
        lists = []
        for b in range(NBLK):
            m = blk == b
            sb_, db_ = s_l[m], d_l[m]
            a_m = sb_ < TA_ROWS
            lists.append((sb_[a_m], db_[a_m] % P, sb_[~a_m] - TA_ROWS,
                          db_[~a_m] % P))
        per_core.append(lists)

    SA = max(1, max((l[0].size + P - 1) // P for ls in per_core for l in ls))
    SB = max(1, max((l[2].size + P - 1) // P for ls in per_core for l in ls))
    S = SA + SB
    meta.update(SA=SA, SB=SB)

    jj = np.arange(P)
    in_maps = []
    for c in range(CORES):
        srcA = np.zeros((NBLK, P, SA * 8), dtype=np.int16)
        srcB = np.zeros((NBLK, P, SB * 8), dtype=np.int16)
        At = np.zeros((NBLK, P, S * P), dtype=np.uint16)
        AT = np.zeros((NBLK, P, S * P), dtype=np.uint16)
        for b in range(NBLK):
            sa, ra, sb_, rb = per_core[c][b]
            nA, nB = sa.size, sb_.size
            a_idx = np.zeros(SA * P, dtype=np.int16)
            a_idx[:nA] = sa
            b_idx = np.zeros(SB * P, dtype=np.int16)
            b_idx[:nB] = sb_
            rel = np.full(S * P, -1, dtype=np.int64)
            rel[:nA] = ra
            rel[SA * P : SA * P + nB] = rb
            rel3 = rel.reshape(S, P)
            # A [p=edge, (t, j)] ; AT [p=j, (t, e)]
            at = (rel3[:, :, None] == jj[None, None, :])      # [t, e, j]
            At[b] = np.where(at.transpose(1, 0, 2).reshape(P, S * P),
                             ONE_BF16, 0)
            AT[b] = np.where(at.transpose(2, 0, 1).reshape(P, S * P),
                             ONE_BF16, 0)
            srcA[b] = _wrap_idx(a_idx)
            srcB[b] = _wrap_idx(b_idx)
        m = {
            "xT": np.roll(xT, -c * NLOC, axis=1),
            "W_ext": W_ext,
            "W2": W2,
            "ident128": np.where(np.eye(P, dtype=bool), ONE_BF16, 0).astype(
                np.uint16
            ),
            "srcA": srcA,
            "srcB": srcB,
            "At": At,
            "AT": AT,
        }
        if meta["has_bias"]:
            m["bias_t"] = np.broadcast_to(bias, (P, OUT_DIM)).copy()
        if meta["has_gamma"]:
            m["gamma_t"] = np.broadcast_to(ln_gamma, (P, OUT_DIM)).copy()
        if meta["has_beta"]:
            m["beta_t"] = np.broadcast_to(ln_beta, (P, OUT_DIM)).copy()
        if meta["has_resb"]:
            m["resb_t"] = np.broadcast_to(res_b, (P, OUT_DIM)).copy()
        in_maps.append(m)

    return meta, in_maps


def build_kernel(meta: dict):
    SA, SB = meta["SA"], meta["SB"]
    S = SA + SB
    SS = S + 1          # +1 self subtile

    nc = bacc.Bacc("TRN2", target_bir_lowering=False, debug=False,
                   num_devices=CORES)

    xT_d = nc.dram_tensor("xT", [IN_DIM, NPAD], FP, kind="ExternalInput")
    Wext_d = nc.dram_tensor("W_ext", [IN_DIM, 264], FP, kind="ExternalInput")
    W2_d = nc.dram_tensor("W2", [IN_DIM, 264], FP, kind="ExternalInput")
    id_d = nc.dram_tensor("ident128", [P, P], BF, kind="ExternalInput")
    srcA_d = nc.dram_tensor("srcA", [NBLK, P, SA * 8], I16, kind="ExternalInput")
    srcB_d = nc.dram_tensor("srcB", [NBLK, P, SB * 8], I16, kind="ExternalInput")
    At_d = nc.dram_tensor("At", [NBLK, P, S * P], BF, kind="ExternalInput")
    AT_d = nc.dram_tensor("AT", [NBLK, P, S * P], BF, kind="ExternalInput")
    opt_in = {}
    for flag, name in [
        ("has_bias", "bias_t"), ("has_gamma", "gamma_t"),
        ("has_beta", "beta_t"), ("has_resb", "resb_t"),
    ]:
        if meta[flag]:
            opt_in[name] = nc.dram_tensor(name, [P, OUT_DIM], FP,
                                          kind="ExternalInput")

    out_d = nc.dram_tensor("out", [NLOC, OUT_DIM], FP, kind="ExternalOutput")

    tabA = nc.dram_tensor("tabA", [TA_ROWS, ROW], BF)
    tabB = nc.dram_tensor("tabB", [TB_ROWS, ROW], BF)
    adst = nc.dram_tensor("adst", [NLOC, ADW], BF)
    ident = nc.dram_tensor("ident", [NLOC, OUT_DIM], FP)
    stage = [
        nc.dram_tensor(f"stage{cch}", [CHUNK_BLKS * P, OUT_DIM], FP)
        for cch in range(NBLK // CHUNK_BLKS)
    ]

    NT = NPAD // P
    XCH = 16

    with tile.TileContext(nc) as tc:
        with tc.tile_pool(name="consts", bufs=1) as consts, \
             tc.tile_pool(name="xchunk", bufs=2) as xchunk, \
             tc.tile_pool(name="p1ps", bufs=2, space="PSUM") as p1ps, \
             tc.tile_pool(name="p1row", bufs=4) as p1row:
            Wext_t = consts.tile([IN_DIM, 264], FP)
            nc.sync.dma_start(Wext_t[:], Wext_d.ap())
            W2_t = consts.tile([IN_DIM, 264], FP)
            nc.sync.dma_start(W2_t[:], W2_d.ap())
            id_t = consts.tile([P, P], BF)
            nc.sync.dma_start(id_t[:], id_d.ap())
            eps_t = consts.tile([P, 1], FP)
            nc.vector.memset(eps_t[:], LN_EPS)
            opt_sb = {}
            for name, dd in opt_in.items():
                t_ = consts.tile([P, OUT_DIM], FP)
                nc.sync.dma_start(t_[:], dd.ap())
                opt_sb[name] = t_

            # ---------------- phase 1: h | a_src | a_dst | residual --------
            for ch in range((NT + XCH - 1) // XCH):
                j0 = ch * XCH
                jn = min(XCH, NT - j0)
                xc = xchunk.tile([P, XCH * P], FP, tag="xc")
                nc.sync.dma_start(
                    xc[:, : jn * P], xT_d.ap()[:, j0 * P : (j0 + jn) * P]
                )
                for t in range(jn):
                    j = j0 + t
                    ps1 = p1ps.tile([P, 264], FP, tag="ps")
                    nc.tensor.matmul(
                        ps1[:], lhsT=xc[:, t * P : (t + 1) * P], rhs=Wext_t[:],
                        start=True, stop=True,
                    )
                    row = p1row.tile([P, RWR], BF, tag="row")
                    nc.scalar.copy(row[:, 0:256], ps1[:, 0:256])
                    nc.scalar.copy(row[:, 256:264], ps1[:, 256:264])
                    nc.vector.tensor_tensor(
                        out=row[:, 264:272], in0=ps1[:, 256:264],
                        in1=row[:, 256:264], op=mybir.AluOpType.subtract,
                    )
                    # rows are 384 wide; only 272 written (tail never read)
                    if j < 256:
                        dst_rows = tabA.ap()[j * P : (j + 1) * P, 0:RWR]
                    else:
                        jb = j - 256
                        dst_rows = tabB.ap()[jb * P : (jb + 1) * P, 0:RWR]
                    nc.sync.dma_start(dst_rows, row[:])
                    if j < NBLK:
                        ps2 = p1ps.tile([P, 264], FP, tag="ps")
                        nc.tensor.matmul(
                            ps2[:], lhsT=xc[:, t * P : (t + 1) * P],
                            rhs=W2_t[:], start=True, stop=True,
                        )
                        ad = p1row.tile([P, ADW], BF, tag="ad")
                        nc.scalar.copy(ad[:, 0:8], ps2[:, 0:8])
                        nc.vector.tensor_tensor(
                            out=ad[:, 8:16], in0=ps2[:, 0:8],
                            in1=ad[:, 0:8], op=mybir.AluOpType.subtract,
                        )
                        nc.sync.dma_start(
                            adst.ap()[j * P : (j + 1) * P, :], ad[:]
                        )
                        idt = p1row.tile([P, OUT_DIM], FP, tag="idt")
                        if meta["has_resb"]:
                            nc.vector.tensor_add(
                                idt[:], ps2[:, 8:264], opt_sb["resb_t"][:]
                            )
                        else:
                            nc.scalar.copy(idt[:], ps2[:, 8:264])
                        nc.sync.dma_start(
                            ident.ap()[j * P : (j + 1) * P, :], idt[:]
                        )

            tc.strict_bb_all_engine_barrier()

            # ------------- phase 2: gather + attention + aggregate ---------
            with tc.tile_pool(name="idx", bufs=3) as idxp, \
                 tc.tile_pool(name="hostA", bufs=3) as hostA, \
                 tc.tile_pool(name="gath", bufs=3) as gath, \
                 tc.tile_pool(name="work", bufs=2) as work, \
                 tc.tile_pool(name="adps", bufs=3, space="PSUM") as adps, \
                 tc.tile_pool(name="aggps", bufs=3, space="PSUM") as aggps, \
                 tc.tile_pool(name="fin", bufs=3) as finp, \
                 tc.tile_pool(name="fin2", bufs=1) as finp2:
                for b in range(NBLK):
                    ia = idxp.tile([P, SA * 8], I16, tag="ia")
                    nc.sync.dma_start(ia[:], srcA_d.ap()[b])
                    ib = idxp.tile([P, SB * 8], I16, tag="ib")
                    nc.sync.dma_start(ib[:], srcB_d.ap()[b])
                    At = hostA.tile([P, S * P], BF, tag="At")
                    nc.sync.dma_start(At[:], At_d.ap()[b])
                    AT = hostA.tile([P, S * P], BF, tag="AT")
                    nc.sync.dma_start(AT[:], AT_d.ap()[b])
                    adblk = idxp.tile([P, ADW], BF, tag="adblk")
                    nc.sync.dma_start(
                        adblk[:], adst.ap()[b * P : (b + 1) * P, :]
                    )

                    G = gath.tile([P, SS, ROW], BF, tag="G")
                    nc.gpsimd.dma_gather(
                        G[:, 0:SA, :], tabA.ap(), ia[:], SA * P, SA * P, ROW,
                        single_packet=SA * P <= 1024,
                    )
                    nc.gpsimd.dma_gather(
                        G[:, SA:S, :], tabB.ap(), ib[:], SB * P, SB * P, ROW,
                        single_packet=SB * P <= 1024,
                    )
                    # self subtile: contiguous table rows of this block
                    nc.sync.dma_start(
                        G[:, S, 0:RWR],
                        tabA.ap()[b * P : (b + 1) * P, 0:RWR],
                    )

                    # a_dst expanded dst->edge via AT one-hot matmuls
                    # (each matmul gets its own PSUM tile; copy into SBUF)
                    pad = work.tile([P, S, ADW], FP, tag="pad")
                    for t in range(S):
                        pps = adps.tile([P, ADW], FP, tag="pps")
                        nc.tensor.matmul(
                            pps[:], lhsT=AT[:, t * P : (t + 1) * P],
                            rhs=adblk[:], start=True, stop=True,
                        )
                        nc.scalar.copy(pad[:, t, :], pps[:])

                    # logits: hi+lo pairs summed
                    es = work.tile([P, SS, 8], FP, tag="es")
                    nc.vector.tensor_add(
                        es[:, 0:S, :], G[:, 0:S, 256:264], G[:, 0:S, 264:272]
                    )
                    es2 = work.tile([P, S, 8], FP, tag="es2")
                    nc.vector.tensor_add(
                        es2[:], pad[:, :, 0:8], pad[:, :, 8:16]
                    )
                    nc.vector.tensor_add(es[:, 0:S, :], es[:, 0:S, :], es2[:])
                    # self logits
                    sads = work.tile([P, 8], FP, tag="sads")
                    nc.vector.tensor_add(
                        sads[:], adblk[:, 0:8], adblk[:, 8:16]
                    )
                    nc.vector.tensor_add(
                        es[:, S, :], G[:, S, 256:264], G[:, S, 264:272]
                    )
                    nc.vector.tensor_add(es[:, S, :], es[:, S, :], sads[:])

                    # w = exp(max(e, 0.2 e)) -> msg[:, :, 256:264] (bf16)
                    lr = work.tile([P, SS, 8], FP, tag="lr")
                    nc.vector.tensor_scalar_mul(lr[:], es[:], NEG_SLOPE)
                    nc.vector.tensor_tensor(
                        out=lr[:], in0=es[:], in1=lr[:], op=mybir.AluOpType.max
                    )
                    msg = work.tile([P, SS, 264], BF, tag="msg")
                    nc.scalar.activation(
                        msg[:, :, 256:264], lr[:],
                        mybir.ActivationFunctionType.Exp,
                    )
                    wrep = work.tile([P, SS, 256], BF, tag="wrep")
                    wsrc = msg[:, :, 256:264]
                    nc.scalar.copy(
                        wrep[:],
                        bass.AP(tensor=wsrc.tensor, offset=wsrc.offset,
                                ap=[wsrc.ap[0], [264, SS], [1, 8], [0, 32]]),
                    )
                    nc.vector.tensor_mul(
                        msg[:, :, 0:256], G[:, :, 0:256], wrep[:]
                    )

                    acc = aggps.tile([P, 264], FP, tag="acc")
                    for t in range(SS):
                        lhsT = id_t[:] if t == S else At[:, t * P : (t + 1) * P]
                        nc.tensor.matmul(
                            acc[:], lhsT=lhsT, rhs=msg[:, t, :],
                            start=(t == 0), stop=(t == SS - 1),
                        )

                    # pass 1 finalize: normalize, stash
                    rec = finp.tile([P, 8], FP, tag="rec")
                    nc.vector.reciprocal(rec[:], acc[:, 256:264])
                    outy = finp.tile([P, OUT_DIM], FP, tag="outy")
                    accv = acc[:, 0:256]
                    nc.vector.tensor_mul(
                        outy[:],
                        bass.AP(tensor=accv.tensor, offset=accv.offset,
                                ap=[accv.ap[0], [32, 8], [1, 32]]),
                        bass.AP(tensor=rec.tensor, offset=rec[:].offset,
                                ap=[rec[:].ap[0], [1, 8], [0, 32]]),
                    )
                    if meta["has_bias"]:
                        nc.vector.tensor_add(
                            outy[:], outy[:], opt_sb["bias_t"][:]
                        )
                    cch, kk = divmod(b, CHUNK_BLKS)
                    nc.sync.dma_start(
                        stage[cch].ap()[kk * P : (kk + 1) * P, :], outy[:]
                    )

                    if kk != CHUNK_BLKS - 1:
                        continue

                    # ---- pass 2 (batched finalize for chunk cch) ----
                    K = CHUNK_BLKS
                    oy = finp2.tile([P, K, OUT_DIM], FP, tag="oy")
                    for k in range(K):
                        nc.sync.dma_start(
                            oy[:, k, :],
                            stage[cch].ap()[k * P : (k + 1) * P, :],
                        )
                    # mean / var per block (manual, batched over K blocks)
                    mean = finp2.tile([P, K], FP, tag="mean")
                    nc.vector.tensor_reduce(
                        mean[:], oy[:], axis=mybir.AxisListType.X,
                        op=mybir.AluOpType.add,
                    )
                    nc.vector.tensor_scalar_mul(mean[:], mean[:], 1.0 / OUT_DIM)
                    sq = finp2.tile([P, K, OUT_DIM], FP, tag="sq")
                    nc.vector.tensor_mul(sq[:], oy[:], oy[:])
                    var = finp2.tile([P, K], FP, tag="var")
                    nc.vector.tensor_reduce(
                        var[:], sq[:], axis=mybir.AxisListType.X,
                        op=mybir.AluOpType.add,
                    )
                    nc.vector.tensor_scalar_mul(var[:], var[:], 1.0 / OUT_DIM)
                    msq = finp2.tile([P, K], FP, tag="msq")
                    nc.vector.tensor_mul(msq[:], mean[:], mean[:])
                    nc.vector.tensor_tensor(
                        out=var[:], in0=var[:], in1=msq[:],
                        op=mybir.AluOpType.subtract,
                    )
                    std = finp2.tile([P, K], FP, tag="std")
                    nc.scalar.activation(
                        std[:], var[:],
                        mybir.ActivationFunctionType.Sqrt, bias=eps_t[:],
                    )
                    rstd = finp2.tile([P, K], FP, tag="rstd")
                    nc.vector.reciprocal(rstd[:], std[:])
                    z = finp2.tile([P, K, OUT_DIM], FP, tag="z")
                    for k in range(K):
                        nc.vector.tensor_scalar(
                            out=z[:, k, :], in0=oy[:, k, :],
                            scalar1=mean[:, k : k + 1],
                            scalar2=rstd[:, k : k + 1],
                            op0=mybir.AluOpType.subtract,
                            op1=mybir.AluOpType.mult,
                        )
                        if meta["has_gamma"]:
                            nc.vector.tensor_mul(
                                z[:, k, :], z[:, k, :], opt_sb["gamma_t"][:]
                            )
                        if meta["has_beta"]:
                            nc.vector.tensor_add(
                                z[:, k, :], z[:, k, :], opt_sb["beta_t"][:]
                            )
                    idt = finp2.tile([P, K, OUT_DIM], FP, tag="idt2")
                    for k in range(K):
                        nc.sync.dma_start(
                            idt[:, k, :],
                            ident.ap()[(cch * K + k) * P : (cch * K + k + 1) * P, :],
                        )
                    nc.vector.tensor_add(z[:], z[:], idt[:])
                    # ELU(z) = max(z, exp(min(z, 0)) - 1)
                    zm = finp2.tile([P, K, OUT_DIM], FP, tag="zm")
                    nc.vector.tensor_scalar_min(zm[:], z[:], 0.0)
                    ez = finp2.tile([P, K, OUT_DIM], FP, tag="ez")
                    nc.scalar.activation(
                        ez[:], zm[:], mybir.ActivationFunctionType.Exp
                    )
                    nc.vector.tensor_scalar_add(ez[:], ez[:], -1.0)
                    fin = finp2.tile([P, K, OUT_DIM], FP, tag="fin")
                    nc.vector.tensor_tensor(
                        out=fin[:], in0=z[:], in1=ez[:],
                        op=mybir.AluOpType.max,
                    )
                    for k in range(K):
                        nc.sync.dma_start(
                            out_d.ap()[(cch * K + k) * P : (cch * K + k + 1) * P, :],
                            fin[:, k, :],
                        )

    nc.compile()
    return nc


_CACHE = {}


def kernel(**inputs) -> np.ndarray:
    meta, in_maps = prepare_host(inputs)
    key = tuple(sorted(meta.items()))
    if key not in _CACHE:
        _CACHE[key] = build_kernel(meta)
    nc = _CACHE[key]
    res = run_bass_kernel_spmd(nc, in_maps, list(range(CORES)))
    full = np.concatenate([res.results[c]["out"] for c in range(CORES)], axis=0)
    return np.ascontiguousarray(full[:N])
